# revision 18
# baseline (speedup 1.0000x reference)
"""Multi-head self-attention (RoPE, causal) on 8 Trainium2 NeuronCores.

Sharding: core c -> (batch = c//2, head-group = c%2 of 8 heads).
Column-parallel wq/wk/wv, row-parallel wo. Each core emits a partial
out^T [f, s]; the host sums the two partials per batch and transposes.

Layouts (all chosen so no on-device transposes are needed):
  XT  [d, s]   (x transposed on host, bf16)
  Q^T/K^T [e, s] per head from matmul(lhsT=wT[d,e], rhs=XT[d,s])
  V   [s, e]   from matmul(lhsT=XT[d,s], rhs=wvT[d,e])
  S^T [j, i] = matmul(lhsT=K^T[e,j], rhs=Q^T[e,i])
  ctx^T [e, i] = matmul(lhsT=V[j,e], rhs=expS^T[j,i])
  out^T [f, s] = matmul(lhsT=woT[d,f], rhs=ctx^T[d,s])

All DRAM inputs are pre-tiled on the host into p-major [128, ...]
layouts so every load is a single dense contiguous DMA. All matmul
operands are bf16 (PSUM accumulation stays fp32); softmax statistics
and RoPE arithmetic stay fp32.

RoPE: head dims de-interleaved on host (even dims -> partitions 0..63,
odd -> 64..127 of each head's Q^T/K^T) by permuting wq/wk rows. The
partner-swap is an SBUF->SBUF DMA partition rotation (2 copies); the
pair signs are folded into the host-precomputed sin table. The
1/sqrt(dk) scale is applied via the Exp activation's scale field.

Softmax: no max-subtraction (scores are O(1)-scaled; fp32 exp is safe).
Causal masking by block-skipping + one 128x128 triangular mask on
diagonal blocks. Row sums: exp tiles are accumulated per i-block into
one fp32 SBUF accumulator on DVE, then a single all-ones [128,128]
matmul per i-block gives the partition-broadcast row sums for free;
normalization multiplies ctx^T by a fast DVE reciprocal of that tile.
"""

import numpy as np
import ml_dtypes

import concourse.bass as bass
import concourse.tile as tile
import concourse.mybir as mybir
from concourse import bacc, bass_utils

F32 = mybir.dt.float32
F32R = mybir.dt.float32r
BF16 = mybir.dt.bfloat16

B = 4
S = 2048
D = 2048
NH = 16
DK = 128
NCORES = 8
HPC = 8            # heads per core
DLOC = HPC * DK    # 1024, local model dims per core
ST = S // 128      # 16 sequence 128-tiles
DT = D // 128      # 16 model-dim 128-tiles
NDT = DLOC // 128  # 8 local model-dim 128-tiles
IB = S // 512      # 4 i-blocks of 512
ROPE_THETA = 10000.0
SCALE = float(1.0 / np.sqrt(DK))

_cache = {}


def build_program():
    if "nc" in _cache:
        return _cache["nc"]

    nc = bacc.Bacc("TRN2", target_bir_lowering=False, debug=False,
                   num_devices=NCORES)

    # all weight/activation inputs are host-pre-tiled p-major: one dense DMA
    xt = nc.dram_tensor("xt", [4, 128, DT, 512], BF16, kind="ExternalInput").ap()
    wq = nc.dram_tensor("wq", [HPC, 128, DT, DK], BF16, kind="ExternalInput").ap()
    wk = nc.dram_tensor("wk", [HPC, 128, DT, DK], BF16, kind="ExternalInput").ap()
    wv = nc.dram_tensor("wv", [2, 128, DT, 512], BF16, kind="ExternalInput").ap()
    wo = nc.dram_tensor("wo", [128, NDT, D], BF16, kind="ExternalInput").ap()
    cct = nc.dram_tensor("cct", [128, S], F32, kind="ExternalInput").ap()
    sst = nc.dram_tensor("sst", [128, S], F32, kind="ExternalInput").ap()
    tri = nc.dram_tensor("tri", [128, 128], BF16, kind="ExternalInput").ap()
    out = nc.dram_tensor("out", [DT, IB, 128, 512], F32,
                         kind="ExternalOutput").ap()

    with tile.TileContext(nc) as tc:
        with (
            tc.tile_pool(name="dram", bufs=1, space="DRAM") as dram_pool,
            tc.tile_pool(name="ctx7", bufs=4) as ctx7_pool,
        ):
            # ctx round-trip layout is read-dense: [ib, 128, head, 512]
            ctx_dram = dram_pool.tile([IB, 128, HPC - 1, 512], BF16)
            ctx7 = _attention_phase(nc, tc, xt, wq, wk, wv, cct, sst,
                                    tri, ctx_dram, ctx7_pool)
            _output_phase(nc, tc, wo, ctx_dram, out, ctx7)

    nc.compile()
    _cache["nc"] = nc
    return nc


def _attention_phase(nc, tc, xt, wq, wk, wv, cct, sst, tri, ctx_dram,
                     ctx7_pool):
    with (
        tc.tile_pool(name="xt", bufs=1) as xt_pool,
        tc.tile_pool(name="vsb", bufs=1) as v_pool,
        tc.tile_pool(name="tabs", bufs=1) as tab_pool,
        tc.tile_pool(name="wqk", bufs=2) as wqk_pool,
        tc.tile_pool(name="qkraw", bufs=2) as raw_pool,
        tc.tile_pool(name="rqk", bufs=2) as rqk_pool,
        tc.tile_pool(name="qk_ps", bufs=2, space="PSUM") as qk_ps_pool,
        tc.tile_pool(name="s_ps", bufs=2, space="PSUM") as s_ps_pool,
    ):
        # ---- resident loads (dense contiguous DMAs, priority order) ----
        def load_wqk(h):
            wq_sb = wqk_pool.tile([128, DT, DK], BF16, tag="wq")
            wk_sb = wqk_pool.tile([128, DT, DK], BF16, tag="wk")
            nc.sync.dma_start(wk_sb[:], wk[h])
            nc.sync.dma_start(wq_sb[:], wq[h])
            return wq_sb, wk_sb

        xt_sb = xt_pool.tile([128, 4, DT, 512], BF16)
        wv_sb = tab_pool.tile([128, 2, DT, 512], BF16, tag="wv")
        cc_sb = tab_pool.tile([128, S], F32, tag="cct")
        ss_sb = tab_pool.tile([128, S], F32, tag="sst")
        tri_sb = tab_pool.tile([128, 128], BF16, tag="tri")
        ones32_sb = tab_pool.tile([128, 128], F32, tag="ones32")
        ones_sb = tab_pool.tile([128, 128], F32R, tag="ones")
        warm_sb = tab_pool.tile([128, 1], F32, tag="warm")

        # HWDGE ring order == arrival order: emit in deadline order. The
        # first proj chunk needs wk(h0) + xt chunk 0 + cc/ss chunk 0 only.
        wk0_sb = wqk_pool.tile([128, DT, DK], BF16, tag="wk")
        nc.sync.dma_start(wk0_sb[:], wk[0])
        nc.sync.dma_start(xt_sb[:, 0, 0:8], xt[0, :, 0:8])
        nc.sync.dma_start(xt_sb[:, 0, 8:16], xt[0, :, 8:16])
        wq0_sb = wqk_pool.tile([128, DT, DK], BF16, tag="wq")
        nc.sync.dma_start(wq0_sb[:], wq[0])
        wqk0 = (wq0_sb, wk0_sb)
        nc.sync.dma_start(cc_sb[:, 0:512], cct[:, 0:512])
        nc.sync.dma_start(ss_sb[:, 0:512], sst[:, 0:512])
        nc.sync.dma_start(tri_sb[:], tri)
        nc.gpsimd.memset(ones32_sb[:], 1.0)
        nc.vector.tensor_copy(ones_sb[:], ones32_sb[:])
        # preload the exp spline tables while DMAs stream
        nc.scalar.activation(warm_sb[:], ones32_sb[:, 0:1],
                             mybir.ActivationFunctionType.Exp)
        nc.sync.dma_start(wv_sb[:, 0], wv[0].rearrange("p d c -> p (d c)"))
        for ch in range(1, 4):
            nc.sync.dma_start(xt_sb[:, ch], xt[ch].rearrange("p d c -> p (d c)"))
            o = ch * 512
            nc.sync.dma_start(cc_sb[:, o:o + 512], cct[:, o:o + 512])
            nc.sync.dma_start(ss_sb[:, o:o + 512], sst[:, o:o + 512])
        nc.sync.dma_start(wv_sb[:, 1], wv[1].rearrange("p d c -> p (d c)"))

        def proj_chunk(w_sb, r_t, ch):
            o = ch * 512
            ps = qk_ps_pool.tile([128, 512], F32, tag="qk_ps")
            for dt in range(DT):
                nc.tensor.matmul(
                    ps[:],
                    w_sb[:, dt, :],
                    xt_sb[:, ch, dt, :],
                    start=(dt == 0), stop=(dt == DT - 1),
                )
            raw = raw_pool.tile([128, 512], BF16, tag="qkraw")
            nc.scalar.copy(raw[:], ps[:])
            # RoPE partner swap: partition rotation by 64 via SBUF->SBUF DMA
            # (pair signs are folded into the host sin table). Issued on the
            # ACT HWDGE ring right after the raw copy: deps already met, and
            # the bulk-input ring can't head-of-line-block it.
            swp = raw_pool.tile([128, 512], BF16, tag="swp")
            nc.scalar.dma_start(swp[0:64, :], raw[64:128, :])
            nc.scalar.dma_start(swp[64:128, :], raw[0:64, :])
            t2 = raw_pool.tile([128, 512], F32, tag="t2")
            nc.gpsimd.tensor_mul(t2[:], swp[:], ss_sb[:, o:o + 512])
            t3 = raw_pool.tile([128, 512], BF16, tag="t3")
            nc.vector.tensor_mul(t3[:], raw[:], cc_sb[:, o:o + 512])
            nc.vector.tensor_add(r_t[:, o:o + 512], t2[:], t3[:])

        # head 0's projection is emitted per-chunk, interleaved with its
        # attention i-blocks (chunk ib is exactly what i-block ib consumes),
        # so DMA-paced chunks don't head-of-line-block ready attention work
        rq0 = rqk_pool.tile([128, S], BF16, tag="rq")
        rk0 = rqk_pool.tile([128, S], BF16, tag="rk")
        proj_chunk(wqk0[1], rk0, 0)
        proj_chunk(wqk0[0], rq0, 0)
        rqk0 = (rq0, rk0)

        # ---- V = x @ wv.T (emitted interleaved with head-0 attention) ----
        v_sb = v_pool.tile([128, ST, DLOC], BF16)

        def emit_v(st, g):
            ch, k = divmod(st, 4)
            v_ps = qk_ps_pool.tile([128, 512], F32, tag="qk_ps")
            for dt in range(DT):
                nc.tensor.matmul(
                    v_ps[:],
                    xt_sb[:, ch, dt, k * 128:(k + 1) * 128],
                    wv_sb[:, g, dt, :],
                    start=(dt == 0), stop=(dt == DT - 1),
                )
            nc.scalar.copy(v_sb[:, st, g * 512:(g + 1) * 512], v_ps[:])

        # ---- per-head attention (+ next head's projection interleaved) ----
        with (
            tc.tile_pool(name="exps", bufs=5) as exp_pool,
            tc.tile_pool(name="pq", bufs=2) as pq_pool,
            tc.tile_pool(name="acc", bufs=1) as acc_pool,
            tc.tile_pool(name="small", bufs=1) as small_pool,
            tc.tile_pool(name="ctxsb", bufs=2) as ctx_sb_pool,
            tc.tile_pool(name="ctx_ps", bufs=2, space="PSUM") as ctx_ps_pool,
            tc.tile_pool(name="rs_ps", bufs=2, space="PSUM") as rs_ps_pool,
        ):
            ctx7 = []
            pending = [None]

            def flush():
                # emit the previous i-block's finalize under fresh PE cover,
                # so its rowsum matmul never serializes PE behind the DVE tree
                if pending[0] is not None:
                    pending[0]()
                    pending[0] = None

            def finalize(acc, rs_ps, ctx_ps, h, ib):
                def fin():
                    nc.tensor.matmul(
                        rs_ps[:],
                        ones_sb[:],
                        acc[:],
                        start=True, stop=True, skip_group_check=True,
                    )
                    recip = small_pool.tile([128, 512], F32, tag="recip")
                    nc.vector.reciprocal_approx_fast(recip[:], rs_ps[:])
                    if h == HPC - 1:
                        ctx_sb = ctx7_pool.tile([128, 512], BF16, tag="c7")
                        ctx7.append(ctx_sb)
                    else:
                        ctx_sb = ctx_sb_pool.tile([128, 512], BF16,
                                                  tag="ctx_sb")
                    nc.vector.tensor_mul(ctx_sb[:], ctx_ps[:], recip[:])
                    if h != HPC - 1:
                        nc.sync.dma_start(ctx_dram[ib][:, h, :], ctx_sb[:])
                return fin

            for h in range(HPC):
                if h == 0:
                    rq, rk = rqk0
                else:
                    wq_sb, wk_sb = load_wqk(h)
                    rq = rqk_pool.tile([128, S], BF16, tag="rq")
                    rk = rqk_pool.tile([128, S], BF16, tag="rk")
                    proj_chunk(wk_sb, rk, 0)
                    flush()
                    proj_chunk(wq_sb, rq, 0)
                    for ch in range(1, 4):
                        proj_chunk(wk_sb, rk, ch)
                        proj_chunk(wq_sb, rq, ch)

                for ib in range(IB):
                    if h == 0:
                        if ib > 0:
                            proj_chunk(wqk0[1], rk, ib)
                            flush()
                            proj_chunk(wqk0[0], rq, ib)
                        # V tiles this i-block needs (g=0), just in time
                        for st in range(4 * ib, 4 * ib + 4):
                            emit_v(st, 0)
                    elif h == 1 and ib == 0:
                        for st in range(4):
                            emit_v(st, 1)
                        flush()
                        for st in range(4, ST):
                            emit_v(st, 1)
                    i0 = ib * 512
                    ctx_ps = ctx_ps_pool.tile([128, 512], F32, tag="ctx_ps")
                    rs_ps = rs_ps_pool.tile([128, 512], F32, tag="rs_ps")
                    acc = acc_pool.tile([128, 512], F32R, tag="acc")
                    njt = 4 * ib + 4
                    es_prev = None
                    pair_pend = None
                    quad_pend = None
                    acc_init = False
                    for jt in range(njt):
                        r = jt - 4 * ib  # >=0 on diagonal blocks
                        lo = 128 * r if r >= 0 else 0
                        s_ps = s_ps_pool.tile([128, 512], F32, tag="s_ps")
                        nc.tensor.matmul(
                            s_ps[:, lo:512],
                            rk[:, jt * 128:(jt + 1) * 128],
                            rq[:, i0 + lo:i0 + 512],
                            start=True, stop=True,
                        )
                        if jt == 0:
                            flush()
                        es = exp_pool.tile([128, 512], BF16, tag="exps")
                        nc.scalar.activation(es[:, lo:512], s_ps[:, lo:512],
                                             mybir.ActivationFunctionType.Exp,
                                             scale=SCALE)
                        if r >= 0:
                            nc.vector.tensor_mul(es[:, lo:lo + 128],
                                                 es[:, lo:lo + 128], tri_sb[:])
                        # row sums: bf16 pair/quad tree, fp32 acc merges (DVE)
                        if r < 0:
                            if es_prev is None:
                                es_prev = es
                            else:
                                pair = pq_pool.tile([128, 512], BF16,
                                                    tag="pair")
                                nc.vector.tensor_add(pair[:], es_prev[:],
                                                     es[:])
                                es_prev = None
                                if pair_pend is None:
                                    pair_pend = pair
                                else:
                                    quad = pq_pool.tile([128, 512], BF16,
                                                        tag="quad")
                                    nc.vector.tensor_add(quad[:],
                                                         pair_pend[:], pair[:])
                                    pair_pend = None
                                    if quad_pend is None:
                                        quad_pend = quad
                                    else:
                                        nc.vector.tensor_add(acc[:],
                                                             quad_pend[:],
                                                             quad[:])
                                        quad_pend = None
                                        acc_init = True
                        elif r == 0:
                            if not acc_init:
                                if quad_pend is not None:
                                    nc.vector.tensor_add(acc[:],
                                                         quad_pend[:], es[:])
                                    quad_pend = None
                                else:
                                    nc.vector.tensor_copy(acc[:], es[:])
                                acc_init = True
                            else:
                                if quad_pend is not None:
                                    nc.vector.tensor_add(acc[:], acc[:],
                                                         quad_pend[:])
                                    quad_pend = None
                                nc.vector.tensor_add(acc[:], acc[:], es[:])
                        else:
                            nc.vector.tensor_add(acc[:, lo:512],
                                                 acc[:, lo:512], es[:, lo:512])
                        nc.tensor.matmul(
                            ctx_ps[:, lo:512],
                            v_sb[:, jt, h * DK:(h + 1) * DK],
                            es[:, lo:512],
                            start=(jt == 0), stop=(jt == njt - 1),
                            skip_group_check=True,
                        )
                    pending[0] = finalize(acc, rs_ps, ctx_ps, h, ib)
            flush()
            return ctx7


def _output_phase(nc, tc, wo, ctx_dram, out, ctx7):
    with (
        tc.tile_pool(name="wos", bufs=1) as wo_pool,
        tc.tile_pool(name="ctxin", bufs=1) as cin_pool,
        tc.tile_pool(name="outsb", bufs=6) as out_pool,
        tc.tile_pool(name="wo_ps", bufs=8, space="PSUM") as wo_ps_pool,
    ):
        # dense reads: all four i-blocks' ctx resident for the ft loop
        cin = cin_pool.tile([128, IB, NDT - 1, 512], BF16)
        nc.sync.dma_start(cin[:, 0], ctx_dram[0])
        wo_sb = wo_pool.tile([128, NDT, D], BF16)
        nc.sync.dma_start(wo_sb[:], wo)
        for sb4 in range(1, IB):
            nc.sync.dma_start(cin[:, sb4], ctx_dram[sb4])
        # ft-major with all 4 i-blocks sharing each wo lhsT load; head 7's
        # contribution comes from SBUF-resident ctx (no DRAM round-trip)
        for ft in range(DT):
            pss = []
            for _ in range(IB):
                wo_ps = wo_ps_pool.tile([128, 512], F32, tag="wo_ps")
                pss.append(wo_ps)
            for dt in range(NDT - 1):
                for sb4 in range(IB):
                    nc.tensor.matmul(
                        pss[sb4][:],
                        wo_sb[:, dt, ft * 128:(ft + 1) * 128],
                        cin[:, sb4, dt, :],
                        start=(dt == 0), stop=False,
                        skip_group_check=True,
                    )
            for sb4 in range(IB):
                nc.tensor.matmul(
                    pss[sb4][:],
                    wo_sb[:, NDT - 1, ft * 128:(ft + 1) * 128],
                    ctx7[sb4][:],
                    start=False, stop=True,
                    skip_group_check=True,
                )
            for sb4 in range(IB):
                osb = out_pool.tile([128, 512], F32, tag="osb")
                nc.scalar.copy(osb[:], pss[sb4][:])
                nc.sync.dma_start(out[ft, sb4], osb[:])


def _tile2(a, p, q):
    """[R, C] -> [R//p, C//q, p, q] contiguous blocks."""
    R, C = a.shape
    return np.ascontiguousarray(
        a.reshape(R // p, p, C // q, q).transpose(0, 2, 1, 3))


def prepare_in_maps(x, wq, wk, wv, wo):
    """Build the 8 per-core input maps (host-side sharding + tables)."""
    x = np.asarray(x, dtype=np.float32)
    wq = np.asarray(wq, dtype=np.float32)
    wk = np.asarray(wk, dtype=np.float32)
    wv = np.asarray(wv, dtype=np.float32)
    wo = np.asarray(wo, dtype=np.float32)
    bf16 = ml_dtypes.bfloat16

    # RoPE tables (fp32, matching the reference's fp32 cos/sin); the pair
    # signs of the rotation are folded into the sin table's top half
    f = np.arange(0, DK, 2, dtype=np.float32) / DK          # 2f/d
    inv_freq = (ROPE_THETA ** (-f)).astype(np.float32)      # [64]
    ang = np.arange(S, dtype=np.float32)[:, None] * inv_freq[None, :]
    cos_t = np.cos(ang).T.astype(np.float32)                # [64, S]
    sin_t = np.sin(ang).T.astype(np.float32)
    cc = np.ascontiguousarray(np.vstack([cos_t, cos_t]))    # [128, S]
    ss = np.ascontiguousarray(np.vstack([-sin_t, sin_t]))

    tri = np.tril(np.ones((128, 128), dtype=np.float32)).T  # tri[j,i]=1 if j<=i
    tri = np.ascontiguousarray(tri).astype(bf16)

    deint = np.concatenate([np.arange(0, DK, 2), np.arange(1, DK, 2)])
    in_maps = []
    for c in range(NCORES):
        bi, g = divmod(c, 2)
        heads = [g * HPC + h for h in range(HPC)]
        qk_rows = np.concatenate([hg * DK + deint for hg in heads])
        v_rows = np.arange(g * DLOC, (g + 1) * DLOC)

        # p-major pre-tiles: [128, DT, cols] per logical unit
        xt_t = _tile2(x[bi].T.astype(bf16), 128, 512)            # [DT,4,128,512]
        xt_t = xt_t.transpose(1, 2, 0, 3)                        # [4,128,DT,512]
        # w*_prep [d, e_loc] -> [DT, HPC, 128, DK] -> [HPC, 128, DT, DK]
        wq_t = _tile2(wq[qk_rows, :].T.astype(bf16), 128, DK).transpose(1, 2, 0, 3)
        wk_t = _tile2(wk[qk_rows, :].T.astype(bf16), 128, DK).transpose(1, 2, 0, 3)
        # wv pre-tiled g-major: [2, 128, DT, 512]
        wv_t = _tile2(wv[v_rows, :].T.astype(bf16), 128, 512).transpose(1, 2, 0, 3)
        # wo: [NDT, 128, D] -> [128, NDT, D]
        wo_t = _tile2(wo.T[v_rows, :].astype(bf16), 128, D)[:, 0].transpose(1, 0, 2)
        in_maps.append({
            "xt": np.ascontiguousarray(xt_t),
            "wq": np.ascontiguousarray(wq_t),
            "wk": np.ascontiguousarray(wk_t),
            "wv": np.ascontiguousarray(wv_t),
            "wo": np.ascontiguousarray(wo_t),
            "cct": cc, "sst": ss,
            "tri": tri,
        })
    return in_maps


def assemble(results):
    out = np.empty((B, S, D), dtype=np.float32)
    for bi in range(B):
        oT = results[2 * bi]["out"] + results[2 * bi + 1]["out"]
        # oT: [DT, IB, 128, 512] -> out^T [f, s]; out[b] = out^T.T
        oT = oT.transpose(0, 2, 1, 3).reshape(D, S)
        out[bi] = oT.T
    return out


def kernel(**inputs):
    nc = build_program()
    in_maps = prepare_in_maps(inputs["x"], inputs["wq"], inputs["wk"],
                              inputs["wv"], inputs["wo"])
    res = bass_utils.run_bass_kernel_spmd(nc, in_maps,
                                          core_ids=list(range(NCORES)))
    return assemble(res.results)


# revision 20
# speedup vs baseline: 1.1852x; 1.1852x over previous
"""Multi-head self-attention (RoPE, causal) on 8 Trainium2 NeuronCores.

Sharding: core c -> (batch = c//2, head-group = c%2 of 8 heads).
Column-parallel wq/wk/wv, row-parallel wo. Each core emits a partial
out^T [f, s]; the host sums the two partials per batch and transposes.

Layouts (all chosen so no on-device transposes are needed):
  XT  [d, s]   (x transposed on host, bf16)
  Q^T/K^T [e, s] per head from matmul(lhsT=wT[d,e], rhs=XT[d,s])
  V   [s, e]   from matmul(lhsT=XT[d,s], rhs=wvT[d,e])
  S^T [j, i] = matmul(lhsT=K^T[e,j], rhs=Q^T[e,i])
  ctx^T [e, i] = matmul(lhsT=V[j,e], rhs=expS^T[j,i])
  out^T [f, s] = matmul(lhsT=woT[d,f], rhs=ctx^T[d,s])

All DRAM inputs are pre-tiled on the host into p-major [128, ...]
layouts so every load is a single dense contiguous DMA. All matmul
operands are bf16 (PSUM accumulation stays fp32); softmax statistics
and RoPE arithmetic stay fp32.

RoPE: head dims de-interleaved on host (even dims -> partitions 0..63,
odd -> 64..127 of each head's Q^T/K^T) by permuting wq/wk rows. The
partner-swap is an SBUF->SBUF DMA partition rotation (2 copies); the
pair signs are folded into the host-precomputed sin table. The
1/sqrt(dk) scale is applied via the Exp activation's scale field.

Softmax: no max-subtraction (scores are O(1)-scaled; fp32 exp is safe).
Causal masking by block-skipping + one 128x128 triangular mask on
diagonal blocks. Row sums: exp tiles are accumulated per i-block into
one fp32 SBUF accumulator on DVE, then a single all-ones [128,128]
matmul per i-block gives the partition-broadcast row sums for free;
normalization multiplies ctx^T by a fast DVE reciprocal of that tile.
"""

import numpy as np
import ml_dtypes

import concourse.bass as bass
import concourse.tile as tile
import concourse.mybir as mybir
from concourse import bacc, bass_utils

F32 = mybir.dt.float32
F32R = mybir.dt.float32r
BF16 = mybir.dt.bfloat16

B = 4
S = 2048
D = 2048
NH = 16
DK = 128
NCORES = 8
HPC = 8            # heads per core
DLOC = HPC * DK    # 1024, local model dims per core
ST = S // 128      # 16 sequence 128-tiles
DT = D // 128      # 16 model-dim 128-tiles
NDT = DLOC // 128  # 8 local model-dim 128-tiles
IB = S // 512      # 4 i-blocks of 512
ROPE_THETA = 10000.0
SCALE = float(1.0 / np.sqrt(DK))

_cache = {}


def build_program():
    if "nc" in _cache:
        return _cache["nc"]

    nc = bacc.Bacc("TRN2", target_bir_lowering=False, debug=False,
                   num_devices=NCORES)

    # all weight/activation inputs are host-pre-tiled p-major: one dense DMA
    xt = nc.dram_tensor("xt", [4, 128, DT, 512], BF16, kind="ExternalInput").ap()
    wq = nc.dram_tensor("wq", [HPC, 128, DT, DK], BF16, kind="ExternalInput").ap()
    wk = nc.dram_tensor("wk", [HPC, 128, DT, DK], BF16, kind="ExternalInput").ap()
    wv = nc.dram_tensor("wv", [2, 128, DT, 512], BF16, kind="ExternalInput").ap()
    wo = nc.dram_tensor("wo", [128, NDT, D], BF16, kind="ExternalInput").ap()
    cct = nc.dram_tensor("cct", [128, S], F32, kind="ExternalInput").ap()
    sst = nc.dram_tensor("sst", [128, S], F32, kind="ExternalInput").ap()
    tri = nc.dram_tensor("tri", [128, 128], BF16, kind="ExternalInput").ap()
    out = nc.dram_tensor("out", [DT, IB, 128, 512], F32,
                         kind="ExternalOutput").ap()

    with tile.TileContext(nc) as tc:
        with (
            tc.tile_pool(name="dram", bufs=1, space="DRAM") as dram_pool,
            tc.tile_pool(name="ctx7", bufs=4) as ctx7_pool,
        ):
            # ctx round-trip layout is read-dense: [ib, 128, head, 512]
            ctx_dram = dram_pool.tile([IB, 128, HPC - 1, 512], BF16)
            ctx7 = _attention_phase(nc, tc, xt, wq, wk, wv, cct, sst,
                                    tri, ctx_dram, ctx7_pool)
            _output_phase(nc, tc, wo, ctx_dram, out, ctx7)

    nc.compile()
    _cache["nc"] = nc
    return nc


def _attention_phase(nc, tc, xt, wq, wk, wv, cct, sst, tri, ctx_dram,
                     ctx7_pool):
    with (
        tc.tile_pool(name="xt", bufs=1) as xt_pool,
        tc.tile_pool(name="vsb", bufs=1) as v_pool,
        tc.tile_pool(name="tabs", bufs=1) as tab_pool,
        tc.tile_pool(name="wqk", bufs=2) as wqk_pool,
        tc.tile_pool(name="qkraw", bufs=2) as raw_pool,
        tc.tile_pool(name="rqk", bufs=2) as rqk_pool,
        tc.tile_pool(name="qk_ps", bufs=2, space="PSUM") as qk_ps_pool,
        tc.tile_pool(name="s_ps", bufs=2, space="PSUM") as s_ps_pool,
    ):
        # ---- resident loads (dense contiguous DMAs, priority order) ----
        def load_wqk(h):
            wq_sb = wqk_pool.tile([128, DT, DK], BF16, tag="wq")
            wk_sb = wqk_pool.tile([128, DT, DK], BF16, tag="wk")
            nc.sync.dma_start(wk_sb[:], wk[h])
            nc.sync.dma_start(wq_sb[:], wq[h])
            return wq_sb, wk_sb

        xt_sb = xt_pool.tile([128, 4, DT, 512], BF16)
        wv_sb = tab_pool.tile([128, 2, DT, 512], BF16, tag="wv")
        cc_sb = tab_pool.tile([128, S], F32, tag="cct")
        ss_sb = tab_pool.tile([128, S], F32, tag="sst")
        tri_sb = tab_pool.tile([128, 128], BF16, tag="tri")
        ones32_sb = tab_pool.tile([128, 128], F32, tag="ones32")
        ones_sb = tab_pool.tile([128, 128], F32R, tag="ones")
        warm_sb = tab_pool.tile([128, 1], F32, tag="warm")

        # HWDGE ring order == arrival order: emit in deadline order. The
        # first proj chunk needs wk(h0) + xt chunk 0 + cc/ss chunk 0 only.
        wk0_sb = wqk_pool.tile([128, DT, DK], BF16, tag="wk")
        nc.sync.dma_start(wk0_sb[:], wk[0])
        nc.sync.dma_start(xt_sb[:, 0, 0:8], xt[0, :, 0:8])
        nc.sync.dma_start(xt_sb[:, 0, 8:16], xt[0, :, 8:16])
        wq0_sb = wqk_pool.tile([128, DT, DK], BF16, tag="wq")
        nc.sync.dma_start(wq0_sb[:], wq[0])
        wqk0 = (wq0_sb, wk0_sb)
        nc.sync.dma_start(cc_sb[:, 0:512], cct[:, 0:512])
        nc.sync.dma_start(ss_sb[:, 0:512], sst[:, 0:512])
        nc.sync.dma_start(tri_sb[:], tri)
        nc.gpsimd.memset(ones32_sb[:], 1.0)
        nc.vector.tensor_copy(ones_sb[:], ones32_sb[:])
        # preload the exp spline tables while DMAs stream
        nc.scalar.activation(warm_sb[:], ones32_sb[:, 0:1],
                             mybir.ActivationFunctionType.Exp)
        nc.sync.dma_start(wv_sb[:, 0], wv[0].rearrange("p d c -> p (d c)"))
        for ch in range(1, 4):
            nc.sync.dma_start(xt_sb[:, ch], xt[ch].rearrange("p d c -> p (d c)"))
            o = ch * 512
            nc.sync.dma_start(cc_sb[:, o:o + 512], cct[:, o:o + 512])
            nc.sync.dma_start(ss_sb[:, o:o + 512], sst[:, o:o + 512])
        nc.sync.dma_start(wv_sb[:, 1], wv[1].rearrange("p d c -> p (d c)"))

        def proj_chunk(w_sb, r_t, ch):
            o = ch * 512
            ps = qk_ps_pool.tile([128, 512], F32, tag="qk_ps")
            for dt in range(DT):
                nc.tensor.matmul(
                    ps[:],
                    w_sb[:, dt, :],
                    xt_sb[:, ch, dt, :],
                    start=(dt == 0), stop=(dt == DT - 1),
                )
            raw = raw_pool.tile([128, 512], BF16, tag="qkraw")
            nc.scalar.copy(raw[:], ps[:])
            # RoPE partner swap: partition rotation by 64 via SBUF->SBUF DMA
            # (pair signs are folded into the host sin table). Issued on the
            # ACT HWDGE ring right after the raw copy: deps already met, and
            # the bulk-input ring can't head-of-line-block it.
            swp = raw_pool.tile([128, 512], BF16, tag="swp")
            nc.scalar.dma_start(swp[0:64, :], raw[64:128, :])
            nc.scalar.dma_start(swp[64:128, :], raw[0:64, :])
            t2 = raw_pool.tile([128, 512], F32, tag="t2")
            nc.gpsimd.tensor_mul(t2[:], swp[:], ss_sb[:, o:o + 512])
            t3 = raw_pool.tile([128, 512], BF16, tag="t3")
            nc.vector.tensor_mul(t3[:], raw[:], cc_sb[:, o:o + 512])
            nc.vector.tensor_add(r_t[:, o:o + 512], t2[:], t3[:])

        # head 0's projection is emitted per-chunk, interleaved with its
        # attention i-blocks (chunk ib is exactly what i-block ib consumes),
        # so DMA-paced chunks don't head-of-line-block ready attention work
        rq0 = rqk_pool.tile([128, S], BF16, tag="rq")
        rk0 = rqk_pool.tile([128, S], BF16, tag="rk")
        proj_chunk(wqk0[1], rk0, 0)
        proj_chunk(wqk0[0], rq0, 0)
        rqk0 = (rq0, rk0)

        # ---- V = x @ wv.T (emitted interleaved with head-0 attention) ----
        v_sb = v_pool.tile([128, ST, DLOC], BF16)

        def emit_v(st, g):
            ch, k = divmod(st, 4)
            v_ps = qk_ps_pool.tile([128, 512], F32, tag="qk_ps")
            for dt in range(DT):
                nc.tensor.matmul(
                    v_ps[:],
                    xt_sb[:, ch, dt, k * 128:(k + 1) * 128],
                    wv_sb[:, g, dt, :],
                    start=(dt == 0), stop=(dt == DT - 1),
                )
            nc.scalar.copy(v_sb[:, st, g * 512:(g + 1) * 512], v_ps[:])

        # ---- per-head attention (+ next head's projection interleaved) ----
        with (
            tc.tile_pool(name="exps", bufs=5) as exp_pool,
            tc.tile_pool(name="pq", bufs=2) as pq_pool,
            tc.tile_pool(name="acc", bufs=1) as acc_pool,
            tc.tile_pool(name="small", bufs=1) as small_pool,
            tc.tile_pool(name="ctxsb", bufs=2) as ctx_sb_pool,
            tc.tile_pool(name="ctx_ps", bufs=2, space="PSUM") as ctx_ps_pool,
            tc.tile_pool(name="rs_ps", bufs=2, space="PSUM") as rs_ps_pool,
        ):
            ctx7 = []
            pending = [None]

            def flush():
                # emit the previous i-block's finalize under fresh PE cover,
                # so its rowsum matmul never serializes PE behind the DVE tree
                if pending[0] is not None:
                    pending[0]()
                    pending[0] = None

            def finalize(acc, rs_ps, ctx_ps, h, ib):
                def fin():
                    nc.tensor.matmul(
                        rs_ps[:],
                        ones_sb[:],
                        acc[:],
                        start=True, stop=True, skip_group_check=True,
                    )
                    recip = small_pool.tile([128, 512], F32, tag="recip")
                    nc.vector.reciprocal_approx_fast(recip[:], rs_ps[:])
                    if h == HPC - 1:
                        ctx_sb = ctx7_pool.tile([128, 512], BF16, tag="c7")
                        ctx7.append(ctx_sb)
                    else:
                        ctx_sb = ctx_sb_pool.tile([128, 512], BF16,
                                                  tag="ctx_sb")
                    nc.vector.tensor_mul(ctx_sb[:], ctx_ps[:], recip[:])
                    if h != HPC - 1:
                        nc.sync.dma_start(ctx_dram[ib][:, h, :], ctx_sb[:])
                return fin

            for h in range(HPC):
                if h == 0:
                    rq, rk = rqk0
                else:
                    wq_sb, wk_sb = load_wqk(h)
                    rq = rqk_pool.tile([128, S], BF16, tag="rq")
                    rk = rqk_pool.tile([128, S], BF16, tag="rk")
                    proj_chunk(wk_sb, rk, 0)
                    proj_chunk(wq_sb, rq, 0)
                    flush()
                    for ch in range(1, 4):
                        proj_chunk(wk_sb, rk, ch)
                        proj_chunk(wq_sb, rq, ch)

                for ib in range(IB):
                    if h == 0:
                        if ib > 0:
                            proj_chunk(wqk0[1], rk, ib)
                            proj_chunk(wqk0[0], rq, ib)
                        # V tiles this i-block needs (g=0), just in time
                        for st in range(4 * ib, 4 * ib + 4):
                            emit_v(st, 0)
                            if st == 4 * ib:
                                flush()
                    elif h == 1 and ib == 0:
                        for st in range(4):
                            emit_v(st, 1)
                        flush()
                        for st in range(4, ST):
                            emit_v(st, 1)
                    i0 = ib * 512
                    ctx_ps = ctx_ps_pool.tile([128, 512], F32, tag="ctx_ps")
                    rs_ps = rs_ps_pool.tile([128, 512], F32, tag="rs_ps")
                    acc = acc_pool.tile([128, 512], F32R, tag="acc")
                    njt = 4 * ib + 4
                    es_prev = None
                    pair_pend = None
                    quad_pend = None
                    acc_init = False
                    for jt in range(njt):
                        r = jt - 4 * ib  # >=0 on diagonal blocks
                        lo = 128 * r if r >= 0 else 0
                        s_ps = s_ps_pool.tile([128, 512], F32, tag="s_ps")
                        nc.tensor.matmul(
                            s_ps[:, lo:512],
                            rk[:, jt * 128:(jt + 1) * 128],
                            rq[:, i0 + lo:i0 + 512],
                            start=True, stop=True,
                        )
                        if jt == 3:
                            flush()
                        es = exp_pool.tile([128, 512], BF16, tag="exps")
                        nc.scalar.activation(es[:, lo:512], s_ps[:, lo:512],
                                             mybir.ActivationFunctionType.Exp,
                                             scale=SCALE)
                        if r >= 0:
                            nc.vector.tensor_mul(es[:, lo:lo + 128],
                                                 es[:, lo:lo + 128], tri_sb[:])
                        # row sums: bf16 pair/quad tree, fp32 acc merges (DVE)
                        if r < 0:
                            if es_prev is None:
                                es_prev = es
                            else:
                                pair = pq_pool.tile([128, 512], BF16,
                                                    tag="pair")
                                nc.vector.tensor_add(pair[:], es_prev[:],
                                                     es[:])
                                es_prev = None
                                if pair_pend is None:
                                    pair_pend = pair
                                else:
                                    quad = pq_pool.tile([128, 512], BF16,
                                                        tag="quad")
                                    nc.vector.tensor_add(quad[:],
                                                         pair_pend[:], pair[:])
                                    pair_pend = None
                                    if quad_pend is None:
                                        quad_pend = quad
                                    else:
                                        nc.vector.tensor_add(acc[:],
                                                             quad_pend[:],
                                                             quad[:])
                                        quad_pend = None
                                        acc_init = True
                        elif r == 0:
                            if not acc_init:
                                if quad_pend is not None:
                                    nc.vector.tensor_add(acc[:],
                                                         quad_pend[:], es[:])
                                    quad_pend = None
                                else:
                                    nc.vector.tensor_copy(acc[:], es[:])
                                acc_init = True
                            else:
                                if quad_pend is not None:
                                    nc.vector.tensor_add(acc[:], acc[:],
                                                         quad_pend[:])
                                    quad_pend = None
                                nc.vector.tensor_add(acc[:], acc[:], es[:])
                        else:
                            nc.vector.tensor_add(acc[:, lo:512],
                                                 acc[:, lo:512], es[:, lo:512])
                        nc.tensor.matmul(
                            ctx_ps[:, lo:512],
                            v_sb[:, jt, h * DK:(h + 1) * DK],
                            es[:, lo:512],
                            start=(jt == 0), stop=(jt == njt - 1),
                            skip_group_check=True,
                        )
                    pending[0] = finalize(acc, rs_ps, ctx_ps, h, ib)
            flush()
            return ctx7


def _output_phase(nc, tc, wo, ctx_dram, out, ctx7):
    with (
        tc.tile_pool(name="wos", bufs=1) as wo_pool,
        tc.tile_pool(name="ctxin", bufs=1) as cin_pool,
        tc.tile_pool(name="outsb", bufs=6) as out_pool,
        tc.tile_pool(name="wo_ps", bufs=8, space="PSUM") as wo_ps_pool,
    ):
        # dense reads: all four i-blocks' ctx resident for the ft loop
        cin = cin_pool.tile([128, IB, NDT - 1, 512], BF16)
        nc.sync.dma_start(cin[:, 0], ctx_dram[0])
        wo_sb = wo_pool.tile([128, NDT, D], BF16)
        nc.sync.dma_start(wo_sb[:], wo)
        for sb4 in range(1, IB):
            nc.sync.dma_start(cin[:, sb4], ctx_dram[sb4])
        # ft-major with all 4 i-blocks sharing each wo lhsT load; head 7's
        # contribution comes from SBUF-resident ctx (no DRAM round-trip)
        for ft in range(DT):
            pss = []
            for _ in range(IB):
                wo_ps = wo_ps_pool.tile([128, 512], F32, tag="wo_ps")
                pss.append(wo_ps)
            for dt in range(NDT - 1):
                for sb4 in range(IB):
                    nc.tensor.matmul(
                        pss[sb4][:],
                        wo_sb[:, dt, ft * 128:(ft + 1) * 128],
                        cin[:, sb4, dt, :],
                        start=(dt == 0), stop=False,
                        skip_group_check=True,
                    )
            for sb4 in range(IB):
                nc.tensor.matmul(
                    pss[sb4][:],
                    wo_sb[:, NDT - 1, ft * 128:(ft + 1) * 128],
                    ctx7[sb4][:],
                    start=False, stop=True,
                    skip_group_check=True,
                )
            for sb4 in range(IB):
                osb = out_pool.tile([128, 512], F32, tag="osb")
                nc.scalar.copy(osb[:], pss[sb4][:])
                nc.sync.dma_start(out[ft, sb4], osb[:])


def _tile2(a, p, q):
    """[R, C] -> [R//p, C//q, p, q] contiguous blocks."""
    R, C = a.shape
    return np.ascontiguousarray(
        a.reshape(R // p, p, C // q, q).transpose(0, 2, 1, 3))


def prepare_in_maps(x, wq, wk, wv, wo):
    """Build the 8 per-core input maps (host-side sharding + tables)."""
    x = np.asarray(x, dtype=np.float32)
    wq = np.asarray(wq, dtype=np.float32)
    wk = np.asarray(wk, dtype=np.float32)
    wv = np.asarray(wv, dtype=np.float32)
    wo = np.asarray(wo, dtype=np.float32)
    bf16 = ml_dtypes.bfloat16

    # RoPE tables (fp32, matching the reference's fp32 cos/sin); the pair
    # signs of the rotation are folded into the sin table's top half
    f = np.arange(0, DK, 2, dtype=np.float32) / DK          # 2f/d
    inv_freq = (ROPE_THETA ** (-f)).astype(np.float32)      # [64]
    ang = np.arange(S, dtype=np.float32)[:, None] * inv_freq[None, :]
    cos_t = np.cos(ang).T.astype(np.float32)                # [64, S]
    sin_t = np.sin(ang).T.astype(np.float32)
    cc = np.ascontiguousarray(np.vstack([cos_t, cos_t]))    # [128, S]
    ss = np.ascontiguousarray(np.vstack([-sin_t, sin_t]))

    tri = np.tril(np.ones((128, 128), dtype=np.float32)).T  # tri[j,i]=1 if j<=i
    tri = np.ascontiguousarray(tri).astype(bf16)

    deint = np.concatenate([np.arange(0, DK, 2), np.arange(1, DK, 2)])
    in_maps = []
    for c in range(NCORES):
        bi, g = divmod(c, 2)
        heads = [g * HPC + h for h in range(HPC)]
        qk_rows = np.concatenate([hg * DK + deint for hg in heads])
        v_rows = np.arange(g * DLOC, (g + 1) * DLOC)

        # p-major pre-tiles: [128, DT, cols] per logical unit
        xt_t = _tile2(x[bi].T.astype(bf16), 128, 512)            # [DT,4,128,512]
        xt_t = xt_t.transpose(1, 2, 0, 3)                        # [4,128,DT,512]
        # w*_prep [d, e_loc] -> [DT, HPC, 128, DK] -> [HPC, 128, DT, DK]
        wq_t = _tile2(wq[qk_rows, :].T.astype(bf16), 128, DK).transpose(1, 2, 0, 3)
        wk_t = _tile2(wk[qk_rows, :].T.astype(bf16), 128, DK).transpose(1, 2, 0, 3)
        # wv pre-tiled g-major: [2, 128, DT, 512]
        wv_t = _tile2(wv[v_rows, :].T.astype(bf16), 128, 512).transpose(1, 2, 0, 3)
        # wo: [NDT, 128, D] -> [128, NDT, D]
        wo_t = _tile2(wo.T[v_rows, :].astype(bf16), 128, D)[:, 0].transpose(1, 0, 2)
        in_maps.append({
            "xt": np.ascontiguousarray(xt_t),
            "wq": np.ascontiguousarray(wq_t),
            "wk": np.ascontiguousarray(wk_t),
            "wv": np.ascontiguousarray(wv_t),
            "wo": np.ascontiguousarray(wo_t),
            "cct": cc, "sst": ss,
            "tri": tri,
        })
    return in_maps


def assemble(results):
    out = np.empty((B, S, D), dtype=np.float32)
    for bi in range(B):
        oT = results[2 * bi]["out"] + results[2 * bi + 1]["out"]
        # oT: [DT, IB, 128, 512] -> out^T [f, s]; out[b] = out^T.T
        oT = oT.transpose(0, 2, 1, 3).reshape(D, S)
        out[bi] = oT.T
    return out


def kernel(**inputs):
    nc = build_program()
    in_maps = prepare_in_maps(inputs["x"], inputs["wq"], inputs["wk"],
                              inputs["wv"], inputs["wo"])
    res = bass_utils.run_bass_kernel_spmd(nc, in_maps,
                                          core_ids=list(range(NCORES)))
    return assemble(res.results)
